# revision 19
# baseline (speedup 1.0000x reference)
"""Nearest-neighbor 2x upsample on 8 trn2 NeuronCores.

Full input  [16, 128, 128, 128] f32  ->  full output [16, 128, 256, 256] f32.
Pure data-parallel: core i handles batches [2i, 2i+2).

Per-core kernel layout:
  - partitions = channel dim (C=128 exactly)
  - tile = one batch x all channels x HB input rows: sbuf [128, HB, 128]
    (contiguous HB*512 B per partition on load)
  - H- and W-duplication both happen on-chip: two DVE copies per tile (one
    per output-row parity) with a broadcast (step-0) source AP write the
    fully upsampled [128, 2*HB, 2*W] tile, which a single fully-contiguous
    DMA store streams to HBM (this is what sustains fabric-rate stores).
  - Loads are issued from the ACT HWDGE queue, stores from the SP queue, so
    a store stalled on its producer copy never head-of-line-blocks later
    loads. All copies run on DVE (2x fp32 SBUF mode beats ACT copies).
"""

import numpy as np

_B, _C, _H, _W = 16, 128, 128, 128
_NCORES = 8
_BPC = _B // _NCORES  # batches per core
_HB = 32              # input rows per tile

_cached = {}


def _build_program():
    import concourse.bacc as bacc
    import concourse.mybir as mybir
    from concourse import tile

    nc = bacc.Bacc("TRN2", target_bir_lowering=False, debug=False)
    x = nc.declare_dram_parameter(
        "input", [_BPC, _C, _H, _W], mybir.dt.float32, isOutput=False
    )
    y = nc.declare_dram_parameter(
        "output", [_BPC, _C, 2 * _H, 2 * _W], mybir.dt.float32, isOutput=True
    )

    with tile.TileContext(nc) as tc:
        with (
            tc.tile_pool(name="in_pool", bufs=3) as in_pool,
            tc.tile_pool(name="out_pool", bufs=2) as out_pool,
        ):
            ti = 0
            for b in range(_BPC):
                for h0 in range(0, _H, _HB):
                    in_sb = in_pool.tile([_C, _HB, _W], mybir.dt.float32)
                    # loads ride the ACT HWDGE queue, stores the SP queue:
                    # a store stalled on its copy can't block later loads.
                    nc.scalar.dma_start(out=in_sb[:], in_=x[b, :, h0 : h0 + _HB, :])

                    out_sb = out_pool.tile([_C, 2 * _HB, 2 * _W], mybir.dt.float32)
                    src = in_sb[:].unsqueeze(-1).broadcast_to([_C, _HB, _W, 2])
                    for di in range(2):
                        dst = out_sb[:, di : 2 * _HB : 2, :].rearrange(
                            "p h (w two) -> p h w two", two=2
                        )
                        nc.vector.tensor_copy(dst, src)
                    ti += 1

                    nc.sync.dma_start(
                        out=y[b, :, 2 * h0 : 2 * (h0 + _HB), :], in_=out_sb[:]
                    )
    nc.finalize()
    return nc


def kernel(input, scale_factor):
    assert int(scale_factor) == 2, f"only scale_factor=2 supported, got {scale_factor}"
    input = np.ascontiguousarray(np.asarray(input, dtype=np.float32))
    assert input.shape == (_B, _C, _H, _W), input.shape

    from concourse.bass_utils import run_bass_kernel_spmd

    if "nc" not in _cached:
        _cached["nc"] = _build_program()
    nc = _cached["nc"]

    in_maps = [
        {"input": np.ascontiguousarray(input[i * _BPC : (i + 1) * _BPC])}
        for i in range(_NCORES)
    ]
    res = run_bass_kernel_spmd(nc, in_maps, list(range(_NCORES)))
    out = np.concatenate([res.results[i]["output"] for i in range(_NCORES)], axis=0)
    return out
